# revision 27
# baseline (speedup 1.0000x reference)
"""Trainium2 Bass kernel for nn_DebedderNeuronGroup_index.

Math (per layer l, with kn=KN[l], ksci=KS[l]*CI[l], i_dim=ksci+1):
    out[b, k, o] = sum_d x[b, off_l + k, d] * W_l[o, d] + b_l[o]
    y[b, S_l + k*ksci + o] = out[b, k, o]          for o <  ksci
    y[b, S_l + kn*ksci + k] = out[b, k, ksci]      (bias column tail block)
The five layers' outputs exactly tile y's 1,422,218 columns, so every
element of y is written exactly once (pure permutation, no accumulation).

Strategy: pure data parallelism over batch (16 per core, 8 cores).
Host pre-packs x per 1024-token chunk as [128, 4*tl] (d-chunk-major,
contiguous 8KB per partition => ~250GB/s chunk loads vs ~100GB/s for
the strided layout) and W as d-interleaved [128, 4*i_dim], all bf16.
Per 128-token subtile: tokens on PSUM partitions, o on the free dim, so
every HBM store is a [tokens, o] tile whose rows are contiguous in y.

Work order: both L1 chunks first (dense 512-wide matmuls warm the PE
HAM clock gate and cover the early HBM read burst), then L2 chunks 0-2,
L0 and L4 mid-stream (their many tiny stores overlap compute), then L3,
with L2's last chunk at the end (so the kernel tail is one small 0.3MB
store instead of L3's 1MB or L0/L4's 32 tiny DMAs).
All x chunks load up front on the SP ring and stay resident; W1/W2 on
the ACT ring ahead of the y stores; W3's 4MB load rides the gpsimd ring
but is gated behind L2-chunk0's first output tile so it can't steal HBM
read bandwidth from the x stream during the warm-up ramp.

The bias column (o == ksci) is folded into the last o-tile for layers
0/1/2/4 (free: their i_dim % 512 != 0); only layer 3 (4096 = 8*512)
keeps the separate M=1 transposed column pass. Folded bias columns are
extracted into col_acc (ACT engine), PE-transposed once at the end,
and stored with 6 batched DMAs.
"""

import numpy as np
import ml_dtypes

import concourse.bass as bass
import concourse.mybir as mybir
from concourse import bacc
from concourse.tile import TileContext
from concourse.bass_utils import run_bass_kernel_spmd

# ---------------------------------------------------------------- constants
N_CORES = 8
B = 128
BPC = B // N_CORES            # batches per core = 16
D = 512
KN = [64, 128, 256, 256, 10]
KSCI = [27, 576, 1152, 4096, 256]
IDIM = [k + 1 for k in KSCI]
START = [0, 1792, 75648, 370816, 1419648]
I_TOTAL = 1422218
TOK = sum(KN)                 # 714 tokens per batch
TOKL = [BPC * k for k in KN]  # tokens per core per layer
XOFF = np.cumsum([0] + TOKL).tolist()   # token offset per layer in xT
NTOK = XOFF[-1]               # 11424
BBOFF = np.cumsum([0] + IDIM).tolist()  # bias-broadcast offset per layer
BBTOT = BBOFF[-1]             # 6112
TLOAD = 1024                  # tokens per x DMA chunk
OTILE = 512                   # matmul moving free dim / PSUM bank
BF16 = mybir.dt.bfloat16
F16 = mybir.dt.float16
F32 = mybir.dt.float32

TS = {l: (128 if KN[l] >= 128 else (128 // KN[l]) * KN[l]) for l in range(5)}
# work items: (layer, chunk t0). Both L1 chunks first: dense 512-wide
# matmuls warm the PE HAM clock gate and give ~16us of covering work
# while the early HBM read burst (x + W2/W3 tables) streams in. L0/L4
# sit mid-stream where their many tiny stores overlap compute; L2's
# last chunk runs last so the kernel tail is one small 0.3MB store.
WORK = ([(1, 0), (1, TLOAD), (2, 0), (2, TLOAD), (2, 2 * TLOAD), (0, 0), (4, 0)]
        + [(3, t) for t in range(0, TOKL[3], TLOAD)]
        + [(2, 3 * TLOAD)])
# col_acc column base per layer (layer 3 uses the transposed pass)
NSUB = {l: (TOKL[l] + TS[l] - 1) // TS[l] for l in (0, 1, 2, 4)}
CBASE = {1: 0, 2: NSUB[1], 0: NSUB[1] + NSUB[2], 4: NSUB[1] + NSUB[2] + NSUB[0]}

_cache = {}
last_results = None


def _build_bass():
    nc = bacc.Bacc(
        "TRN2", target_bir_lowering=False, debug=False, num_devices=N_CORES
    )
    xP = nc.declare_dram_parameter("xP", [128, 4 * NTOK], BF16, isOutput=False)
    WT = [
        nc.declare_dram_parameter(f"WT{l}", [128, 4 * IDIM[l]], BF16, isOutput=False)
        for l in range(5)
    ]
    BB = nc.declare_dram_parameter("BB", [128, BBTOT], BF16, isOutput=False)
    BCOL = nc.declare_dram_parameter("BCOL", [1, 8], F32, isOutput=False)
    IDN = nc.declare_dram_parameter("IDN", [128, 128], F16, isOutput=False)
    y = nc.declare_dram_parameter("y", [BPC, I_TOTAL], F16, isOutput=True)

    with TileContext(nc) as tc:
        with (
            tc.tile_pool(name="wt", bufs=1) as wt_pool,
            tc.tile_pool(name="bias", bufs=1) as bias_pool,
            tc.tile_pool(name="x", bufs=10) as x_pool,
            tc.tile_pool(name="out", bufs=3) as out_pool,
            tc.tile_pool(name="outs", bufs=8) as outs_pool,
            tc.tile_pool(name="ocol", bufs=4) as ocol_pool,
            tc.tile_pool(name="ps", bufs=7, space="PSUM") as ps_pool,
            tc.tile_pool(name="pst", bufs=1, space="PSUM") as pst_pool,
        ):
            bb = bias_pool.tile([128, BBTOT], BF16, tag="bb")
            idn = bias_pool.tile([128, 128], F16, tag="idn")
            col_acc = bias_pool.tile([128, 64], F16, tag="cacc")
            acc_t = bias_pool.tile([64, 128], F16, tag="accT")
            bcol = bias_pool.tile([1, 8], F32, tag="bcol")
            nc.gpsimd.memset(col_acc[:, :], 0.0)

            wt3 = {}

            def load_w(l, eng):
                t = wt_pool.tile([128, 4 * IDIM[l]], BF16, tag=f"wt{l}")
                eng.dma_start(out=t[:], in_=WT[l][:, :])
                wt3[l] = t[:].rearrange("p (c o) -> p c o", c=4)

            # W1 gates the first matmul; W1+W2 finish on the ACT HW ring
            # (~12us) before the y stores start queueing behind them.
            load_w(1, nc.scalar)
            load_w(2, nc.scalar)
            # gpsimd SW ring, consumption order. Few large DMAs: every DMA
            # instruction occupies a completion-semaphore slot that is
            # recycled across queues, and slot reuse creates false
            # cross-queue ordering, so instruction count matters.
            nc.gpsimd.dma_start(out=bcol[:], in_=BCOL[:, :])
            nc.gpsimd.dma_start(
                out=bb[:, : BBOFF[3]], in_=BB[:, : BBOFF[3]]
            )  # bias for layers 0-2
            load_w(0, nc.gpsimd)
            load_w(4, nc.gpsimd)
            nc.gpsimd.dma_start(out=idn[:], in_=IDN[:, :])
            # W3 (4MB) + layer-3/4 bias are deferred until L2-chunk0's
            # first output tile exists (below) so their loads don't steal
            # HBM read bandwidth during the warm-up ramp.
            wt3_tile = wt_pool.tile([128, 4 * IDIM[3]], BF16, tag="wt3")
            wt3[3] = wt3_tile[:].rearrange("p (c o) -> p c o", c=4)

            # All x chunks up front, in work order. Each chunk is
            # host-packed contiguous ([128, 4*tl], d-chunk-major with
            # stride tl). The first chunk is split so the first matmuls
            # only wait on 256 tokens; two L2 chunks go via the ACT ring
            # to spread early HBM read demand.
            xts = {}
            for n, (l, t0) in enumerate(WORK):
                tl = min(TLOAD, TOKL[l] - t0)
                xt = x_pool.tile([128, 4 * TLOAD], BF16, tag="xt")
                fo = 4 * (XOFF[l] + t0)
                xt3 = xt[:, : 4 * tl].rearrange("p (c t) -> p c t", c=4)
                src3 = xP[:, fo : fo + 4 * tl].rearrange("p (c t) -> p c t", c=4)
                nc.sync.dma_start(out=xt3[:, :, :], in_=src3[:, :, :])
                xts[(l, t0)] = xt3

            for l, t0 in WORK:
                kn, ksci, idim = KN[l], KSCI[l], IDIM[l]
                ocols = idim if l != 3 else ksci
                otiles = [
                    (o0, min(OTILE, ocols - o0)) for o0 in range(0, ocols, OTILE)
                ]
                y_main = y[:, START[l] : START[l] + kn * ksci].rearrange(
                    "b (k o) -> b k o", o=ksci
                )
                y_col = y[:, START[l] + kn * ksci : START[l] + kn * ksci + kn]
                ts = TS[l]
                tl = min(TLOAD, TOKL[l] - t0)
                xt3 = xts[(l, t0)]
                for s0 in range(0, tl, ts):
                    sl = min(ts, tl - s0)          # tokens in subtile
                    tok = t0 + s0                  # layer-token index
                    b0 = tok // kn                 # first batch
                    nb = max(1, sl // kn)          # batches in subtile
                    k0 = tok - b0 * kn             # first k
                    if l == 3:
                        ob = out_pool.tile([128, 4096], F16, tag="ob")
                    else:
                        ob = outs_pool.tile([128, 1216], F16, tag="obs")
                    for oi, (o0, no) in enumerate(otiles):
                        ps = ps_pool.tile([128, OTILE], F32, tag="ps")
                        for dc in range(4):
                            nc.tensor.matmul(
                                out=ps[:sl, :no],
                                lhsT=xt3[:, dc, s0 : s0 + sl],
                                rhs=wt3[l][:, dc, o0 : o0 + no],
                                start=(dc == 0),
                                stop=(dc == 3),
                            )
                        nc.vector.tensor_add(
                            out=ob[:sl, o0 : o0 + no],
                            in0=ps[:sl, :no],
                            in1=bb[:sl, BBOFF[l] + o0 : BBOFF[l] + o0 + no],
                        )
                    if l != 3:
                        c = CBASE[l] + tok // ts
                        nc.scalar.copy(
                            out=col_acc[:sl, c : c + 1],
                            in_=ob[:sl, ksci : ksci + 1],
                        )
                    if (l, t0, s0) == (2, 0, 0):
                        # Defer W3 + late-bias loads until this tile exists.
                        # The copies write one element of the destination
                        # tiles, so the DMAs (same-range writers) are
                        # ordered after them — a real dependency, since the
                        # Tile scheduler ignores emission order.
                        nc.gpsimd.tensor_copy(
                            out=wt3_tile[0:1, 0:1], in_=ob[0:1, 0:1]
                        )
                        nc.gpsimd.dma_start(out=wt3_tile[:], in_=WT[3][:, :])
                        nc.gpsimd.tensor_copy(
                            out=bb[0:1, BBOFF[3] : BBOFF[3] + 1], in_=ob[0:1, 0:1]
                        )
                        nc.gpsimd.dma_start(
                            out=bb[:, BBOFF[3] :], in_=BB[:, BBOFF[3] :]
                        )  # bias for layers 3-4
                    # store: [nb, nk, ksci] rows contiguous in y, one DMA
                    nk = min(kn, sl)
                    nc.scalar.dma_start(
                        out=y_main[b0 : b0 + nb, k0 : k0 + nk, :],
                        in_=ob[: nb * nk, :ksci],
                    )
                if l == 3:
                    # transposed bias-column pass: [1, token] rows
                    for c0 in range(0, tl, OTILE):
                        cl = min(OTILE, tl - c0)
                        pc = ps_pool.tile([128, OTILE], F32, tag="ps")
                        for dc in range(4):
                            nc.tensor.matmul(
                                out=pc[:1, :cl],
                                lhsT=wt3[3][:, dc, ksci : ksci + 1],
                                rhs=xt3[:, dc, c0 : c0 + cl],
                                start=(dc == 0),
                                stop=(dc == 3),
                            )
                        oc = ocol_pool.tile([1, OTILE], F16, tag="oc")
                        nc.any.tensor_scalar_add(
                            out=oc[:1, :cl],
                            in0=pc[:1, :cl],
                            scalar1=bcol[0:1, 3:4],
                        )
                        cb0 = (t0 + c0) // kn
                        nc.sync.dma_start(
                            out=y_col[cb0 : cb0 + cl // kn, :],
                            in_=oc[0:1, :cl],
                        )

            # transpose col_acc -> acc_t and store folded bias columns
            pst = pst_pool.tile([64, 128], F16, tag="pst")
            nc.tensor.transpose(out=pst[:, :], in_=col_acc[:, 0:64], identity=idn[:])
            nc.any.tensor_copy(out=acc_t[:, :], in_=pst[:, :])

            def col_view(l):
                s = START[l] + KN[l] * KSCI[l]
                return y[:, s : s + KN[l]]

            # L1: row b <-> batch b             [16,128] <- [16,128]
            nc.sync.dma_start(out=col_view(1), in_=acc_t[CBASE[1] : CBASE[1] + 16, :])
            # L2: rows 2b,2b+1 <-> batch b      [16,256] <- [32,128]
            nc.sync.dma_start(out=col_view(2), in_=acc_t[CBASE[2] : CBASE[2] + 32, :])
            # L0: row halves <-> batches        [16,64] <- [8,128]
            nc.sync.dma_start(out=col_view(0), in_=acc_t[CBASE[0] : CBASE[0] + 8, :])
            # L4: row 0 tokens 0-119 (b 0-11), row 1 tokens 0-39 (b 12-15)
            c4 = col_view(4)
            nc.sync.dma_start(out=c4[0:12, :], in_=acc_t[CBASE[4] : CBASE[4] + 1, 0:120])
            nc.sync.dma_start(out=c4[12:16, :], in_=acc_t[CBASE[4] + 1 : CBASE[4] + 2, 0:40])
    nc.compile()
    return nc


def _prep_inputs(inputs):
    x = np.asarray(inputs["x"], dtype=np.float32)
    xb = x.astype(ml_dtypes.bfloat16)
    in_maps = []
    # shared across cores
    shared = {}
    for l in range(5):
        W = np.asarray(inputs[f"W{l}"], dtype=np.float32)  # [IDIM, D]
        # d-interleaved: packed[p, c*IDIM+o] = W[o, c*128+p]
        wp = W.T.astype(ml_dtypes.bfloat16)                # [D, IDIM]
        wp = wp.reshape(4, 128, IDIM[l]).transpose(1, 0, 2).reshape(128, -1)
        shared[f"WT{l}"] = np.ascontiguousarray(wp)
    bbvec = np.concatenate(
        [np.asarray(inputs[f"b{l}"], dtype=np.float32) for l in range(5)]
    )
    shared["BB"] = np.ascontiguousarray(
        np.broadcast_to(bbvec.astype(ml_dtypes.bfloat16), (128, BBTOT))
    )
    bcol = np.zeros((1, 8), np.float32)
    for l in range(5):
        bcol[0, l] = np.asarray(inputs[f"b{l}"], dtype=np.float32)[KSCI[l]]
    shared["BCOL"] = bcol
    shared["IDN"] = np.eye(128, dtype=np.float16)
    off = np.cumsum([0] + KN).tolist()
    for c in range(N_CORES):
        xc = xb[c * BPC : (c + 1) * BPC]  # [16, 714, 512] bf16
        parts = [
            np.transpose(xc[:, off[l] : off[l] + KN[l]], (2, 0, 1)).reshape(D, -1)
            for l in range(5)
        ]
        xT = np.concatenate(parts, axis=1)  # [512, 11424] d-major
        # per-chunk contiguous packing: [128, 4*tl] blocks, d-chunk-major
        blocks = []
        for l in range(5):
            for t0 in range(0, TOKL[l], TLOAD):
                tl = min(TLOAD, TOKL[l] - t0)
                blk = xT[:, XOFF[l] + t0 : XOFF[l] + t0 + tl]
                blocks.append(
                    blk.reshape(4, 128, tl).transpose(1, 0, 2).reshape(128, -1)
                )
        in_maps.append({"xP": np.ascontiguousarray(np.concatenate(blocks, axis=1)),
                        **shared})
    return in_maps


def kernel(**inputs):
    global last_results
    if "nc" not in _cache:
        _cache["nc"] = _build_bass()
    nc = _cache["nc"]
    in_maps = _prep_inputs(inputs)
    res = run_bass_kernel_spmd(nc, in_maps, list(range(N_CORES)))
    last_results = res
    y = np.concatenate(
        [res.results[c]["y"].astype(np.float32) for c in range(N_CORES)], axis=0
    )
    return y


# revision 28
# speedup vs baseline: 1.0089x; 1.0089x over previous
"""Trainium2 Bass kernel for nn_DebedderNeuronGroup_index.

Math (per layer l, with kn=KN[l], ksci=KS[l]*CI[l], i_dim=ksci+1):
    out[b, k, o] = sum_d x[b, off_l + k, d] * W_l[o, d] + b_l[o]
    y[b, S_l + k*ksci + o] = out[b, k, o]          for o <  ksci
    y[b, S_l + kn*ksci + k] = out[b, k, ksci]      (bias column tail block)
The five layers' outputs exactly tile y's 1,422,218 columns, so every
element of y is written exactly once (pure permutation, no accumulation).

Strategy: pure data parallelism over batch (16 per core, 8 cores).
Host pre-packs x per 1024-token chunk as [128, 4*tl] (d-chunk-major,
contiguous 8KB per partition => ~250GB/s chunk loads vs ~100GB/s for
the strided layout) and W as d-interleaved [128, 4*i_dim], all bf16.
Per 128-token subtile: tokens on PSUM partitions, o on the free dim, so
every HBM store is a [tokens, o] tile whose rows are contiguous in y.

Work order: both L1 chunks first (dense 512-wide matmuls warm the PE
HAM clock gate and cover the early HBM read burst), then L2 chunks 0-2,
L0 and L4 mid-stream (their many tiny stores overlap compute), then L3,
with L2's last chunk at the end (so the kernel tail is one small 0.3MB
store instead of L3's 1MB or L0/L4's 32 tiny DMAs).
All x chunks load up front on the SP ring and stay resident; W1/W2 on
the ACT ring ahead of the y stores; W3's 4MB load rides the gpsimd ring
but is gated behind L2-chunk0's first output tile so it can't steal HBM
read bandwidth from the x stream during the warm-up ramp.

The bias column (o == ksci) is folded into the last o-tile for layers
0/1/2/4 (free: their i_dim % 512 != 0); only layer 3 (4096 = 8*512)
keeps the separate M=1 transposed column pass. Folded bias columns are
extracted into col_acc (ACT engine), PE-transposed once at the end,
and stored with 6 batched DMAs.
"""

import numpy as np
import ml_dtypes

import concourse.bass as bass
import concourse.mybir as mybir
from concourse import bacc
from concourse.tile import TileContext
from concourse.bass_utils import run_bass_kernel_spmd

# ---------------------------------------------------------------- constants
N_CORES = 8
B = 128
BPC = B // N_CORES            # batches per core = 16
D = 512
KN = [64, 128, 256, 256, 10]
KSCI = [27, 576, 1152, 4096, 256]
IDIM = [k + 1 for k in KSCI]
START = [0, 1792, 75648, 370816, 1419648]
I_TOTAL = 1422218
TOK = sum(KN)                 # 714 tokens per batch
TOKL = [BPC * k for k in KN]  # tokens per core per layer
XOFF = np.cumsum([0] + TOKL).tolist()   # token offset per layer in xT
NTOK = XOFF[-1]               # 11424
BBOFF = np.cumsum([0] + IDIM).tolist()  # bias-broadcast offset per layer
BBTOT = BBOFF[-1]             # 6112
TLOAD = 1024                  # tokens per x DMA chunk
OTILE = 512                   # matmul moving free dim / PSUM bank
BF16 = mybir.dt.bfloat16
F16 = mybir.dt.float16
F32 = mybir.dt.float32

TS = {l: (128 if KN[l] >= 128 else (128 // KN[l]) * KN[l]) for l in range(5)}
# work items: (layer, chunk t0). Both L1 chunks first: dense 512-wide
# matmuls warm the PE HAM clock gate and give ~16us of covering work
# while the early HBM read burst (x + W2/W3 tables) streams in. L0/L4
# sit mid-stream where their many tiny stores overlap compute; L2's
# last chunk runs last so the kernel tail is one small 0.3MB store.
WORK = ([(1, 0), (1, TLOAD), (2, 0), (2, TLOAD), (2, 2 * TLOAD), (0, 0), (4, 0)]
        + [(3, t) for t in range(0, TOKL[3], TLOAD)]
        + [(2, 3 * TLOAD)])
# col_acc column base per layer (layer 3 uses the transposed pass)
NSUB = {l: (TOKL[l] + TS[l] - 1) // TS[l] for l in (0, 1, 2, 4)}
CBASE = {1: 0, 2: NSUB[1], 0: NSUB[1] + NSUB[2], 4: NSUB[1] + NSUB[2] + NSUB[0]}

_cache = {}
last_results = None


def _build_bass():
    nc = bacc.Bacc(
        "TRN2", target_bir_lowering=False, debug=False, num_devices=N_CORES
    )
    xP = nc.declare_dram_parameter("xP", [128, 4 * NTOK], BF16, isOutput=False)
    WT = [
        nc.declare_dram_parameter(f"WT{l}", [128, 4 * IDIM[l]], BF16, isOutput=False)
        for l in range(5)
    ]
    BB = nc.declare_dram_parameter("BB", [128, BBTOT], BF16, isOutput=False)
    BCOL = nc.declare_dram_parameter("BCOL", [1, 8], F32, isOutput=False)
    IDN = nc.declare_dram_parameter("IDN", [128, 128], F16, isOutput=False)
    y = nc.declare_dram_parameter("y", [BPC, I_TOTAL], F16, isOutput=True)

    with TileContext(nc) as tc:
        with (
            tc.tile_pool(name="wt", bufs=1) as wt_pool,
            tc.tile_pool(name="bias", bufs=1) as bias_pool,
            tc.tile_pool(name="x", bufs=10) as x_pool,
            tc.tile_pool(name="out", bufs=4) as out_pool,
            tc.tile_pool(name="outs", bufs=8) as outs_pool,
            tc.tile_pool(name="ocol", bufs=4) as ocol_pool,
            tc.tile_pool(name="ps", bufs=7, space="PSUM") as ps_pool,
            tc.tile_pool(name="pst", bufs=1, space="PSUM") as pst_pool,
        ):
            bb = bias_pool.tile([128, BBTOT], BF16, tag="bb")
            idn = bias_pool.tile([128, 128], F16, tag="idn")
            col_acc = bias_pool.tile([128, 64], F16, tag="cacc")
            acc_t = bias_pool.tile([64, 128], F16, tag="accT")
            bcol = bias_pool.tile([1, 8], F32, tag="bcol")
            nc.gpsimd.memset(col_acc[:, :], 0.0)

            wt3 = {}

            def load_w(l, eng):
                t = wt_pool.tile([128, 4 * IDIM[l]], BF16, tag=f"wt{l}")
                eng.dma_start(out=t[:], in_=WT[l][:, :])
                wt3[l] = t[:].rearrange("p (c o) -> p c o", c=4)

            # W1 gates the first matmul; W1+W2 finish on the ACT HW ring
            # (~12us) before the y stores start queueing behind them.
            load_w(1, nc.scalar)
            load_w(2, nc.scalar)
            # gpsimd SW ring, consumption order. Few large DMAs: every DMA
            # instruction occupies a completion-semaphore slot that is
            # recycled across queues, and slot reuse creates false
            # cross-queue ordering, so instruction count matters.
            nc.gpsimd.dma_start(out=bcol[:], in_=BCOL[:, :])
            nc.gpsimd.dma_start(
                out=bb[:, : BBOFF[3]], in_=BB[:, : BBOFF[3]]
            )  # bias for layers 0-2
            load_w(0, nc.gpsimd)
            load_w(4, nc.gpsimd)
            nc.gpsimd.dma_start(out=idn[:], in_=IDN[:, :])
            # W3 (4MB) + layer-3/4 bias are deferred until L2-chunk0's
            # first output tile exists (below) so their loads don't steal
            # HBM read bandwidth during the warm-up ramp.
            wt3_tile = wt_pool.tile([128, 4 * IDIM[3]], BF16, tag="wt3")
            wt3[3] = wt3_tile[:].rearrange("p (c o) -> p c o", c=4)

            # All x chunks up front, in work order. Each chunk is
            # host-packed contiguous ([128, 4*tl], d-chunk-major with
            # stride tl). The first chunk is split so the first matmuls
            # only wait on 256 tokens; two L2 chunks go via the ACT ring
            # to spread early HBM read demand.
            xts = {}
            for n, (l, t0) in enumerate(WORK):
                tl = min(TLOAD, TOKL[l] - t0)
                xt = x_pool.tile([128, 4 * TLOAD], BF16, tag="xt")
                fo = 4 * (XOFF[l] + t0)
                xt3 = xt[:, : 4 * tl].rearrange("p (c t) -> p c t", c=4)
                src3 = xP[:, fo : fo + 4 * tl].rearrange("p (c t) -> p c t", c=4)
                nc.sync.dma_start(out=xt3[:, :, :], in_=src3[:, :, :])
                xts[(l, t0)] = xt3

            for l, t0 in WORK:
                kn, ksci, idim = KN[l], KSCI[l], IDIM[l]
                ocols = idim if l != 3 else ksci
                otiles = [
                    (o0, min(OTILE, ocols - o0)) for o0 in range(0, ocols, OTILE)
                ]
                y_main = y[:, START[l] : START[l] + kn * ksci].rearrange(
                    "b (k o) -> b k o", o=ksci
                )
                y_col = y[:, START[l] + kn * ksci : START[l] + kn * ksci + kn]
                ts = TS[l]
                tl = min(TLOAD, TOKL[l] - t0)
                xt3 = xts[(l, t0)]
                for s0 in range(0, tl, ts):
                    sl = min(ts, tl - s0)          # tokens in subtile
                    tok = t0 + s0                  # layer-token index
                    b0 = tok // kn                 # first batch
                    nb = max(1, sl // kn)          # batches in subtile
                    k0 = tok - b0 * kn             # first k
                    if l == 3:
                        ob = out_pool.tile([128, 4096], F16, tag="ob")
                    else:
                        ob = outs_pool.tile([128, 1216], F16, tag="obs")
                    for oi, (o0, no) in enumerate(otiles):
                        ps = ps_pool.tile([128, OTILE], F32, tag="ps")
                        for dc in range(4):
                            nc.tensor.matmul(
                                out=ps[:sl, :no],
                                lhsT=xt3[:, dc, s0 : s0 + sl],
                                rhs=wt3[l][:, dc, o0 : o0 + no],
                                start=(dc == 0),
                                stop=(dc == 3),
                            )
                        nc.vector.tensor_add(
                            out=ob[:sl, o0 : o0 + no],
                            in0=ps[:sl, :no],
                            in1=bb[:sl, BBOFF[l] + o0 : BBOFF[l] + o0 + no],
                        )
                    if l != 3:
                        c = CBASE[l] + tok // ts
                        nc.scalar.copy(
                            out=col_acc[:sl, c : c + 1],
                            in_=ob[:sl, ksci : ksci + 1],
                        )
                    if (l, t0, s0) == (2, 0, 0):
                        # Defer W3 + late-bias loads until this tile exists.
                        # The copies write one element of the destination
                        # tiles, so the DMAs (same-range writers) are
                        # ordered after them — a real dependency, since the
                        # Tile scheduler ignores emission order.
                        nc.gpsimd.tensor_copy(
                            out=wt3_tile[0:1, 0:1], in_=ob[0:1, 0:1]
                        )
                        nc.gpsimd.dma_start(out=wt3_tile[:], in_=WT[3][:, :])
                        nc.gpsimd.tensor_copy(
                            out=bb[0:1, BBOFF[3] : BBOFF[3] + 1], in_=ob[0:1, 0:1]
                        )
                        nc.gpsimd.dma_start(
                            out=bb[:, BBOFF[3] :], in_=BB[:, BBOFF[3] :]
                        )  # bias for layers 3-4
                    # store: [nb, nk, ksci] rows contiguous in y, one DMA
                    nk = min(kn, sl)
                    nc.scalar.dma_start(
                        out=y_main[b0 : b0 + nb, k0 : k0 + nk, :],
                        in_=ob[: nb * nk, :ksci],
                    )
                if l == 3:
                    # transposed bias-column pass: [1, token] rows
                    for c0 in range(0, tl, OTILE):
                        cl = min(OTILE, tl - c0)
                        pc = ps_pool.tile([128, OTILE], F32, tag="ps")
                        for dc in range(4):
                            nc.tensor.matmul(
                                out=pc[:1, :cl],
                                lhsT=wt3[3][:, dc, ksci : ksci + 1],
                                rhs=xt3[:, dc, c0 : c0 + cl],
                                start=(dc == 0),
                                stop=(dc == 3),
                            )
                        oc = ocol_pool.tile([1, OTILE], F16, tag="oc")
                        nc.any.tensor_scalar_add(
                            out=oc[:1, :cl],
                            in0=pc[:1, :cl],
                            scalar1=bcol[0:1, 3:4],
                        )
                        cb0 = (t0 + c0) // kn
                        nc.sync.dma_start(
                            out=y_col[cb0 : cb0 + cl // kn, :],
                            in_=oc[0:1, :cl],
                        )

            # transpose col_acc -> acc_t and store folded bias columns
            pst = pst_pool.tile([64, 128], F16, tag="pst")
            nc.tensor.transpose(out=pst[:, :], in_=col_acc[:, 0:64], identity=idn[:])
            nc.any.tensor_copy(out=acc_t[:, :], in_=pst[:, :])

            def col_view(l):
                s = START[l] + KN[l] * KSCI[l]
                return y[:, s : s + KN[l]]

            # L1: row b <-> batch b             [16,128] <- [16,128]
            nc.sync.dma_start(out=col_view(1), in_=acc_t[CBASE[1] : CBASE[1] + 16, :])
            # L2: rows 2b,2b+1 <-> batch b      [16,256] <- [32,128]
            nc.sync.dma_start(out=col_view(2), in_=acc_t[CBASE[2] : CBASE[2] + 32, :])
            # L0: row halves <-> batches        [16,64] <- [8,128]
            nc.sync.dma_start(out=col_view(0), in_=acc_t[CBASE[0] : CBASE[0] + 8, :])
            # L4: row 0 tokens 0-119 (b 0-11), row 1 tokens 0-39 (b 12-15)
            c4 = col_view(4)
            nc.sync.dma_start(out=c4[0:12, :], in_=acc_t[CBASE[4] : CBASE[4] + 1, 0:120])
            nc.sync.dma_start(out=c4[12:16, :], in_=acc_t[CBASE[4] + 1 : CBASE[4] + 2, 0:40])
    nc.compile()
    return nc


def _prep_inputs(inputs):
    x = np.asarray(inputs["x"], dtype=np.float32)
    xb = x.astype(ml_dtypes.bfloat16)
    in_maps = []
    # shared across cores
    shared = {}
    for l in range(5):
        W = np.asarray(inputs[f"W{l}"], dtype=np.float32)  # [IDIM, D]
        # d-interleaved: packed[p, c*IDIM+o] = W[o, c*128+p]
        wp = W.T.astype(ml_dtypes.bfloat16)                # [D, IDIM]
        wp = wp.reshape(4, 128, IDIM[l]).transpose(1, 0, 2).reshape(128, -1)
        shared[f"WT{l}"] = np.ascontiguousarray(wp)
    bbvec = np.concatenate(
        [np.asarray(inputs[f"b{l}"], dtype=np.float32) for l in range(5)]
    )
    shared["BB"] = np.ascontiguousarray(
        np.broadcast_to(bbvec.astype(ml_dtypes.bfloat16), (128, BBTOT))
    )
    bcol = np.zeros((1, 8), np.float32)
    for l in range(5):
        bcol[0, l] = np.asarray(inputs[f"b{l}"], dtype=np.float32)[KSCI[l]]
    shared["BCOL"] = bcol
    shared["IDN"] = np.eye(128, dtype=np.float16)
    off = np.cumsum([0] + KN).tolist()
    for c in range(N_CORES):
        xc = xb[c * BPC : (c + 1) * BPC]  # [16, 714, 512] bf16
        parts = [
            np.transpose(xc[:, off[l] : off[l] + KN[l]], (2, 0, 1)).reshape(D, -1)
            for l in range(5)
        ]
        xT = np.concatenate(parts, axis=1)  # [512, 11424] d-major
        # per-chunk contiguous packing: [128, 4*tl] blocks, d-chunk-major
        blocks = []
        for l in range(5):
            for t0 in range(0, TOKL[l], TLOAD):
                tl = min(TLOAD, TOKL[l] - t0)
                blk = xT[:, XOFF[l] + t0 : XOFF[l] + t0 + tl]
                blocks.append(
                    blk.reshape(4, 128, tl).transpose(1, 0, 2).reshape(128, -1)
                )
        in_maps.append({"xP": np.ascontiguousarray(np.concatenate(blocks, axis=1)),
                        **shared})
    return in_maps


def kernel(**inputs):
    global last_results
    if "nc" not in _cache:
        _cache["nc"] = _build_bass()
    nc = _cache["nc"]
    in_maps = _prep_inputs(inputs)
    res = run_bass_kernel_spmd(nc, in_maps, list(range(N_CORES)))
    last_results = res
    y = np.concatenate(
        [res.results[c]["y"].astype(np.float32) for c in range(N_CORES)], axis=0
    )
    return y


# revision 33
# speedup vs baseline: 1.0539x; 1.0446x over previous
"""Trainium2 Bass kernel for nn_DebedderNeuronGroup_index.

Math (per layer l, with kn=KN[l], ksci=KS[l]*CI[l], i_dim=ksci+1):
    out[b, k, o] = sum_d x[b, off_l + k, d] * W_l[o, d] + b_l[o]
    y[b, S_l + k*ksci + o] = out[b, k, o]          for o <  ksci
    y[b, S_l + kn*ksci + k] = out[b, k, ksci]      (bias column tail block)
The five layers' outputs exactly tile y's 1,422,218 columns, so every
element of y is written exactly once (pure permutation, no accumulation).

Strategy: pure data parallelism over batch (16 per core, 8 cores).
Host pre-packs x per 1024-token chunk as [128, 4*tl] (d-chunk-major,
contiguous 8KB per partition => ~250GB/s chunk loads vs ~100GB/s for
the strided layout) and W as d-interleaved [128, 4*i_dim], all bf16.
Per 128-token subtile: tokens on PSUM partitions, o on the free dim, so
every HBM store is a [tokens, o] tile whose rows are contiguous in y.

Work order: both L1 chunks first (dense 512-wide matmuls warm the PE
HAM clock gate and cover the early HBM read burst), then L2 chunks 0-2,
L0 and L4 mid-stream (their many tiny stores overlap compute), then L3,
with L2's last chunk at the end (so the kernel tail is one small 0.3MB
store instead of L3's 1MB or L0/L4's 32 tiny DMAs).
All x chunks load up front on the SP ring and stay resident; W1/W2 on
the ACT ring ahead of the y stores; W3's 4MB load rides the gpsimd ring
but is gated behind L2-chunk0's first output tile so it can't steal HBM
read bandwidth from the x stream during the warm-up ramp.

The bias column (o == ksci) is folded into the last o-tile for layers
0/1/2/4 (free: their i_dim % 512 != 0); only layer 3 (4096 = 8*512)
keeps the separate M=1 transposed column pass. Folded bias columns are
extracted into col_acc (ACT engine), PE-transposed once at the end,
and stored with 6 batched DMAs.
"""

import numpy as np
import ml_dtypes

import concourse.bass as bass
import concourse.mybir as mybir
from concourse import bacc
from concourse.tile import TileContext
from concourse.bass_utils import run_bass_kernel_spmd

# ---------------------------------------------------------------- constants
N_CORES = 8
B = 128
BPC = B // N_CORES            # batches per core = 16
D = 512
KN = [64, 128, 256, 256, 10]
KSCI = [27, 576, 1152, 4096, 256]
IDIM = [k + 1 for k in KSCI]
START = [0, 1792, 75648, 370816, 1419648]
I_TOTAL = 1422218
TOK = sum(KN)                 # 714 tokens per batch
TOKL = [BPC * k for k in KN]  # tokens per core per layer
XOFF = np.cumsum([0] + TOKL).tolist()   # token offset per layer in xT
NTOK = XOFF[-1]               # 11424
BBOFF = np.cumsum([0] + IDIM).tolist()  # bias-broadcast offset per layer
BBTOT = BBOFF[-1]             # 6112
TLOAD = 1024                  # tokens per x DMA chunk
OTILE = 512                   # matmul moving free dim / PSUM bank
BF16 = mybir.dt.bfloat16
F16 = mybir.dt.float16
F32 = mybir.dt.float32

TS = {l: (128 if KN[l] >= 128 else (128 // KN[l]) * KN[l]) for l in range(5)}
# work items: (layer, chunk t0). Both L1 chunks first: dense 512-wide
# matmuls warm the PE HAM clock gate and give ~16us of covering work
# while the early HBM read burst (x + W2/W3 tables) streams in. L0/L4
# sit mid-stream where their many tiny stores overlap compute; L2's
# last chunk runs last so the kernel tail is one small 0.3MB store.
WORK = ([(1, 0), (1, TLOAD), (2, 0), (2, TLOAD), (2, 2 * TLOAD), (0, 0), (4, 0)]
        + [(3, t) for t in range(0, TOKL[3], TLOAD)]
        + [(2, 3 * TLOAD)])
# col_acc column base per layer (layer 3 uses the transposed pass)
NSUB = {l: (TOKL[l] + TS[l] - 1) // TS[l] for l in (0, 1, 2, 4)}
CBASE = {1: 0, 2: NSUB[1], 0: NSUB[1] + NSUB[2], 4: NSUB[1] + NSUB[2] + NSUB[0]}

_cache = {}
last_results = None


def _build_bass():
    nc = bacc.Bacc(
        "TRN2", target_bir_lowering=False, debug=False, num_devices=N_CORES
    )
    xP = nc.declare_dram_parameter("xP", [128, 4 * NTOK], BF16, isOutput=False)
    WT = [
        nc.declare_dram_parameter(f"WT{l}", [128, 4 * IDIM[l]], BF16, isOutput=False)
        for l in range(5)
    ]
    BB = nc.declare_dram_parameter("BB", [128, BBTOT], BF16, isOutput=False)
    BCOL = nc.declare_dram_parameter("BCOL", [1, 8], F32, isOutput=False)
    IDN = nc.declare_dram_parameter("IDN", [128, 128], F16, isOutput=False)
    y = nc.declare_dram_parameter("y", [BPC, I_TOTAL], F16, isOutput=True)

    with TileContext(nc) as tc:
        with (
            tc.tile_pool(name="wt", bufs=1) as wt_pool,
            tc.tile_pool(name="bias", bufs=1) as bias_pool,
            tc.tile_pool(name="x", bufs=12) as x_pool,
            tc.tile_pool(name="out", bufs=4) as out_pool,
            tc.tile_pool(name="ocol", bufs=4) as ocol_pool,
            tc.tile_pool(name="ps", bufs=7, space="PSUM") as ps_pool,
            tc.tile_pool(name="pst", bufs=1, space="PSUM") as pst_pool,
        ):
            bb = bias_pool.tile([128, BBTOT], BF16, tag="bb")
            idn = bias_pool.tile([128, 128], F16, tag="idn")
            col_acc = bias_pool.tile([128, 64], F16, tag="cacc")
            acc_t = bias_pool.tile([64, 128], F16, tag="accT")
            bcol = bias_pool.tile([1, 8], F32, tag="bcol")
            gate = bias_pool.tile([1, 8], F16, tag="gate")
            nc.gpsimd.memset(col_acc[:, :], 0.0)

            wt3 = {}

            def load_w(l, eng):
                t = wt_pool.tile([128, 4 * IDIM[l]], BF16, tag=f"wt{l}")
                eng.dma_start(out=t[:], in_=WT[l][:, :])
                wt3[l] = t[:].rearrange("p (c o) -> p c o", c=4)

            def load_bb(l):
                nc.gpsimd.dma_start(
                    out=bb[:, BBOFF[l] : BBOFF[l] + IDIM[l]],
                    in_=BB[:, BBOFF[l] : BBOFF[l] + IDIM[l]],
                )

            # W1 gates the first matmul; W1+W2 finish on the ACT HW ring
            # (~12us) before the y stores start queueing behind them.
            load_w(1, nc.scalar)
            load_w(2, nc.scalar)
            # gpsimd SW ring: bias slices + small tables in consumption
            # order; W3's 4MB lands mid-kernel, well before layer 3.
            nc.gpsimd.dma_start(out=bcol[:], in_=BCOL[:, :])
            load_bb(1)
            load_w(0, nc.gpsimd)
            load_bb(0)
            load_w(4, nc.gpsimd)
            load_bb(4)
            load_bb(2)

            # All x chunks up front, in work order. Each chunk is
            # host-packed contiguous ([128, 4*tl], d-chunk-major with
            # stride tl). The first chunk is split so the first matmuls
            # only wait on 256 tokens; two L2 chunks go via the ACT ring
            # to spread early HBM read demand.
            xts = {}
            for n, (l, t0) in enumerate(WORK):
                tl = min(TLOAD, TOKL[l] - t0)
                xt = x_pool.tile([128, 4 * TLOAD], BF16, tag="xt")
                fo = 4 * (XOFF[l] + t0)
                xt3 = xt[:, : 4 * tl].rearrange("p (c t) -> p c t", c=4)
                src3 = xP[:, fo : fo + 4 * tl].rearrange("p (c t) -> p c t", c=4)
                nc.sync.dma_start(out=xt3[:, :, :], in_=src3[:, :, :])
                xts[(l, t0)] = xt3

            for l, t0 in WORK:
                kn, ksci, idim = KN[l], KSCI[l], IDIM[l]
                ocols = idim if l != 3 else ksci
                otiles = [
                    (o0, min(OTILE, ocols - o0)) for o0 in range(0, ocols, OTILE)
                ]
                y_main = y[:, START[l] : START[l] + kn * ksci].rearrange(
                    "b (k o) -> b k o", o=ksci
                )
                y_col = y[:, START[l] + kn * ksci : START[l] + kn * ksci + kn]
                ts = TS[l]
                tl = min(TLOAD, TOKL[l] - t0)
                xt3 = xts[(l, t0)]
                for s0 in range(0, tl, ts):
                    sl = min(ts, tl - s0)          # tokens in subtile
                    tok = t0 + s0                  # layer-token index
                    b0 = tok // kn                 # first batch
                    nb = max(1, sl // kn)          # batches in subtile
                    k0 = tok - b0 * kn             # first k
                    ob = out_pool.tile([128, 4096], F16, tag="ob")
                    for oi, (o0, no) in enumerate(otiles):
                        ps = ps_pool.tile([128, OTILE], F32, tag="ps")
                        for dc in range(4):
                            nc.tensor.matmul(
                                out=ps[:sl, :no],
                                lhsT=xt3[:, dc, s0 : s0 + sl],
                                rhs=wt3[l][:, dc, o0 : o0 + no],
                                start=(dc == 0),
                                stop=(dc == 3),
                            )
                        nc.vector.tensor_add(
                            out=ob[:sl, o0 : o0 + no],
                            in0=ps[:sl, :no],
                            in1=bb[:sl, BBOFF[l] + o0 : BBOFF[l] + o0 + no],
                        )
                    if l != 3:
                        c = CBASE[l] + tok // ts
                        nc.scalar.copy(
                            out=col_acc[:sl, c : c + 1],
                            in_=ob[:sl, ksci : ksci + 1],
                        )
                    if (l, t0, s0) == (2, 0, 0):
                        # release the gated W3 load now that the warm-up
                        # ramp's read burst is past
                        nc.gpsimd.tensor_copy(out=gate[0:1, 0:1], in_=ob[0:1, 0:1])
                        load_w(3, nc.gpsimd)
                        load_bb(3)
                        nc.gpsimd.dma_start(out=idn[:], in_=IDN[:, :])
                    # store per batch: [nk, ksci] rows contiguous in y
                    nk = min(kn, sl)
                    for bi in range(nb):
                        nc.scalar.dma_start(
                            out=y_main[b0 + bi, k0 : k0 + nk, :],
                            in_=ob[bi * nk : bi * nk + nk, :ksci],
                        )
                if l == 3:
                    # transposed bias-column pass: [1, token] rows
                    for c0 in range(0, tl, OTILE):
                        cl = min(OTILE, tl - c0)
                        pc = ps_pool.tile([128, OTILE], F32, tag="ps")
                        for dc in range(4):
                            nc.tensor.matmul(
                                out=pc[:1, :cl],
                                lhsT=wt3[3][:, dc, ksci : ksci + 1],
                                rhs=xt3[:, dc, c0 : c0 + cl],
                                start=(dc == 0),
                                stop=(dc == 3),
                            )
                        oc = ocol_pool.tile([1, OTILE], F16, tag="oc")
                        nc.any.tensor_scalar_add(
                            out=oc[:1, :cl],
                            in0=pc[:1, :cl],
                            scalar1=bcol[0:1, 3:4],
                        )
                        cb0 = (t0 + c0) // kn
                        for bi in range(cl // kn):
                            nc.sync.dma_start(
                                out=y_col[cb0 + bi, :],
                                in_=oc[0:1, bi * kn : (bi + 1) * kn],
                            )

            # transpose col_acc -> acc_t and store folded bias columns
            pst = pst_pool.tile([64, 128], F16, tag="pst")
            nc.tensor.transpose(out=pst[:, :], in_=col_acc[:, 0:64], identity=idn[:])
            nc.any.tensor_copy(out=acc_t[:, :], in_=pst[:, :])

            def col_view(l):
                s = START[l] + KN[l] * KSCI[l]
                return y[:, s : s + KN[l]]

            # L1: row b <-> batch b             [16,128] <- [16,128]
            nc.sync.dma_start(out=col_view(1), in_=acc_t[CBASE[1] : CBASE[1] + 16, :])
            # L2: rows 2b,2b+1 <-> batch b      [16,256] <- [32,128]
            nc.sync.dma_start(out=col_view(2), in_=acc_t[CBASE[2] : CBASE[2] + 32, :])
            # L0: row halves <-> batches        [16,64] <- [8,128]
            nc.sync.dma_start(out=col_view(0), in_=acc_t[CBASE[0] : CBASE[0] + 8, :])
            # L4: row 0 tokens 0-119 (b 0-11), row 1 tokens 0-39 (b 12-15)
            c4 = col_view(4)
            nc.sync.dma_start(out=c4[0:12, :], in_=acc_t[CBASE[4] : CBASE[4] + 1, 0:120])
            nc.sync.dma_start(out=c4[12:16, :], in_=acc_t[CBASE[4] + 1 : CBASE[4] + 2, 0:40])
    nc.compile()
    return nc


def _prep_inputs(inputs):
    x = np.asarray(inputs["x"], dtype=np.float32)
    xb = x.astype(ml_dtypes.bfloat16)
    in_maps = []
    # shared across cores
    shared = {}
    for l in range(5):
        W = np.asarray(inputs[f"W{l}"], dtype=np.float32)  # [IDIM, D]
        # d-interleaved: packed[p, c*IDIM+o] = W[o, c*128+p]
        wp = W.T.astype(ml_dtypes.bfloat16)                # [D, IDIM]
        wp = wp.reshape(4, 128, IDIM[l]).transpose(1, 0, 2).reshape(128, -1)
        shared[f"WT{l}"] = np.ascontiguousarray(wp)
    bbvec = np.concatenate(
        [np.asarray(inputs[f"b{l}"], dtype=np.float32) for l in range(5)]
    )
    shared["BB"] = np.ascontiguousarray(
        np.broadcast_to(bbvec.astype(ml_dtypes.bfloat16), (128, BBTOT))
    )
    bcol = np.zeros((1, 8), np.float32)
    for l in range(5):
        bcol[0, l] = np.asarray(inputs[f"b{l}"], dtype=np.float32)[KSCI[l]]
    shared["BCOL"] = bcol
    shared["IDN"] = np.eye(128, dtype=np.float16)
    off = np.cumsum([0] + KN).tolist()
    for c in range(N_CORES):
        xc = xb[c * BPC : (c + 1) * BPC]  # [16, 714, 512] bf16
        parts = [
            np.transpose(xc[:, off[l] : off[l] + KN[l]], (2, 0, 1)).reshape(D, -1)
            for l in range(5)
        ]
        xT = np.concatenate(parts, axis=1)  # [512, 11424] d-major
        # per-chunk contiguous packing: [128, 4*tl] blocks, d-chunk-major
        blocks = []
        for l in range(5):
            for t0 in range(0, TOKL[l], TLOAD):
                tl = min(TLOAD, TOKL[l] - t0)
                blk = xT[:, XOFF[l] + t0 : XOFF[l] + t0 + tl]
                blocks.append(
                    blk.reshape(4, 128, tl).transpose(1, 0, 2).reshape(128, -1)
                )
        in_maps.append({"xP": np.ascontiguousarray(np.concatenate(blocks, axis=1)),
                        **shared})
    return in_maps


def kernel(**inputs):
    global last_results
    if "nc" not in _cache:
        _cache["nc"] = _build_bass()
    nc = _cache["nc"]
    in_maps = _prep_inputs(inputs)
    res = run_bass_kernel_spmd(nc, in_maps, list(range(N_CORES)))
    last_results = res
    y = np.concatenate(
        [res.results[c]["y"].astype(np.float32) for c in range(N_CORES)], axis=0
    )
    return y


# revision 35
# speedup vs baseline: 1.0625x; 1.0081x over previous
"""Trainium2 Bass kernel for nn_DebedderNeuronGroup_index.

Math (per layer l, with kn=KN[l], ksci=KS[l]*CI[l], i_dim=ksci+1):
    out[b, k, o] = sum_d x[b, off_l + k, d] * W_l[o, d] + b_l[o]
    y[b, S_l + k*ksci + o] = out[b, k, o]          for o <  ksci
    y[b, S_l + kn*ksci + k] = out[b, k, ksci]      (bias column tail block)
The five layers' outputs exactly tile y's 1,422,218 columns, so every
element of y is written exactly once (pure permutation, no accumulation).

Strategy: pure data parallelism over batch (16 per core, 8 cores).
Host pre-packs x per 1024-token chunk as [128, 4*tl] (d-chunk-major,
contiguous 8KB per partition => ~250GB/s chunk loads vs ~100GB/s for
the strided layout) and W as d-interleaved [128, 4*i_dim], all bf16.
Per 128-token subtile: tokens on PSUM partitions, o on the free dim, so
every HBM store is a [tokens, o] tile whose rows are contiguous in y.

Work order: both L1 chunks first (dense 512-wide matmuls warm the PE
HAM clock gate and cover the early HBM read burst), then L2 chunks 0-2,
L0 and L4 mid-stream (their many tiny stores overlap compute), then L3,
with L2's last chunk at the end (so the kernel tail is one small 0.3MB
store instead of L3's 1MB or L0/L4's 32 tiny DMAs).
All x chunks load up front on the SP ring and stay resident; W1/W2 on
the ACT ring ahead of the y stores; W3's 4MB load rides the gpsimd ring
but is gated behind L2-chunk0's first output tile so it can't steal HBM
read bandwidth from the x stream during the warm-up ramp.

The bias column (o == ksci) is folded into the last o-tile for layers
0/1/2/4 (free: their i_dim % 512 != 0); only layer 3 (4096 = 8*512)
keeps the separate M=1 transposed column pass. Folded bias columns are
extracted into col_acc (ACT engine), PE-transposed once at the end,
and stored with 6 batched DMAs.
"""

import numpy as np
import ml_dtypes

import concourse.bass as bass
import concourse.mybir as mybir
from concourse import bacc
from concourse.tile import TileContext
from concourse.bass_utils import run_bass_kernel_spmd

# ---------------------------------------------------------------- constants
N_CORES = 8
B = 128
BPC = B // N_CORES            # batches per core = 16
D = 512
KN = [64, 128, 256, 256, 10]
KSCI = [27, 576, 1152, 4096, 256]
IDIM = [k + 1 for k in KSCI]
START = [0, 1792, 75648, 370816, 1419648]
I_TOTAL = 1422218
TOK = sum(KN)                 # 714 tokens per batch
TOKL = [BPC * k for k in KN]  # tokens per core per layer
XOFF = np.cumsum([0] + TOKL).tolist()   # token offset per layer in xT
NTOK = XOFF[-1]               # 11424
BBOFF = np.cumsum([0] + IDIM).tolist()  # bias-broadcast offset per layer
BBTOT = BBOFF[-1]             # 6112
TLOAD = 1024                  # tokens per x DMA chunk
OTILE = 512                   # matmul moving free dim / PSUM bank
BF16 = mybir.dt.bfloat16
F16 = mybir.dt.float16
F32 = mybir.dt.float32

TS = {l: (128 if KN[l] >= 128 else (128 // KN[l]) * KN[l]) for l in range(5)}
# work items: (layer, chunk t0). Both L1 chunks first: dense 512-wide
# matmuls warm the PE HAM clock gate and give ~16us of covering work
# while the early HBM read burst (x + W2/W3 tables) streams in. L0/L4
# sit mid-stream where their many tiny stores overlap compute; L2's
# last chunk runs last so the kernel tail is one small 0.3MB store.
WORK = ([(1, 0), (1, TLOAD), (2, 0), (2, TLOAD), (2, 2 * TLOAD), (0, 0), (4, 0)]
        + [(3, t) for t in range(0, TOKL[3], TLOAD)]
        + [(2, 3 * TLOAD)])
# col_acc column base per layer (layer 3 uses the transposed pass)
NSUB = {l: (TOKL[l] + TS[l] - 1) // TS[l] for l in (0, 1, 2, 4)}
CBASE = {1: 0, 2: NSUB[1], 0: NSUB[1] + NSUB[2], 4: NSUB[1] + NSUB[2] + NSUB[0]}

_cache = {}
last_results = None


def _build_bass():
    nc = bacc.Bacc(
        "TRN2", target_bir_lowering=False, debug=False, num_devices=N_CORES
    )
    xP = nc.declare_dram_parameter("xP", [128, 4 * NTOK], BF16, isOutput=False)
    WT = [
        nc.declare_dram_parameter(f"WT{l}", [128, 4 * IDIM[l]], BF16, isOutput=False)
        for l in range(5)
    ]
    BB = nc.declare_dram_parameter("BB", [128, BBTOT], BF16, isOutput=False)
    BCOL = nc.declare_dram_parameter("BCOL", [1, 8], F32, isOutput=False)
    IDN = nc.declare_dram_parameter("IDN", [128, 128], F16, isOutput=False)
    y = nc.declare_dram_parameter("y", [BPC, I_TOTAL], F16, isOutput=True)

    with TileContext(nc) as tc:
        with (
            tc.tile_pool(name="wt", bufs=1) as wt_pool,
            tc.tile_pool(name="bias", bufs=1) as bias_pool,
            tc.tile_pool(name="x", bufs=12) as x_pool,
            tc.tile_pool(name="out", bufs=4) as out_pool,
            tc.tile_pool(name="ocol", bufs=4) as ocol_pool,
            tc.tile_pool(name="ps", bufs=7, space="PSUM") as ps_pool,
            tc.tile_pool(name="pst", bufs=1, space="PSUM") as pst_pool,
        ):
            bb = bias_pool.tile([128, BBTOT], BF16, tag="bb")
            idn = bias_pool.tile([128, 128], F16, tag="idn")
            col_acc = bias_pool.tile([128, 64], F16, tag="cacc")
            acc_t = bias_pool.tile([64, 128], F16, tag="accT")
            bcol = bias_pool.tile([1, 8], F32, tag="bcol")
            gate = bias_pool.tile([1, 8], F16, tag="gate")
            nc.gpsimd.memset(col_acc[:, :], 0.0)

            wt3 = {}

            def load_w(l, eng):
                t = wt_pool.tile([128, 4 * IDIM[l]], BF16, tag=f"wt{l}")
                eng.dma_start(out=t[:], in_=WT[l][:, :])
                wt3[l] = t[:].rearrange("p (c o) -> p c o", c=4)

            def load_bb(l):
                nc.gpsimd.dma_start(
                    out=bb[:, BBOFF[l] : BBOFF[l] + IDIM[l]],
                    in_=BB[:, BBOFF[l] : BBOFF[l] + IDIM[l]],
                )

            # W1 gates the first matmul; W1+W2 finish on the ACT HW ring
            # (~12us) before the y stores start queueing behind them.
            load_w(1, nc.scalar)
            load_w(2, nc.scalar)
            # gpsimd SW ring: bias slices + small tables in consumption
            # order; W3's 4MB lands mid-kernel, well before layer 3.
            nc.gpsimd.dma_start(out=bcol[:], in_=BCOL[:, :])
            load_bb(1)
            load_w(0, nc.gpsimd)
            load_bb(0)
            load_w(4, nc.gpsimd)
            load_bb(4)
            load_bb(2)
            # W3's tile exists up front; its 4MB DMA is released below via
            # a write into the tile (WAW ordering — the Tile scheduler
            # ignores emission order, so this is the only way to defer it).
            wt3_tile = wt_pool.tile([128, 4 * IDIM[3]], BF16, tag="wt3")
            wt3[3] = wt3_tile[:].rearrange("p (c o) -> p c o", c=4)

            # All x chunks up front, in work order. Each chunk is
            # host-packed contiguous ([128, 4*tl], d-chunk-major with
            # stride tl). The first chunk is split so the first matmuls
            # only wait on 256 tokens; two L2 chunks go via the ACT ring
            # to spread early HBM read demand.
            xts = {}
            for n, (l, t0) in enumerate(WORK):
                tl = min(TLOAD, TOKL[l] - t0)
                xt = x_pool.tile([128, 4 * TLOAD], BF16, tag="xt")
                fo = 4 * (XOFF[l] + t0)
                xt3 = xt[:, : 4 * tl].rearrange("p (c t) -> p c t", c=4)
                src3 = xP[:, fo : fo + 4 * tl].rearrange("p (c t) -> p c t", c=4)
                nc.sync.dma_start(out=xt3[:, :, :], in_=src3[:, :, :])
                xts[(l, t0)] = xt3

            for l, t0 in WORK:
                kn, ksci, idim = KN[l], KSCI[l], IDIM[l]
                ocols = idim if l != 3 else ksci
                otiles = [
                    (o0, min(OTILE, ocols - o0)) for o0 in range(0, ocols, OTILE)
                ]
                y_main = y[:, START[l] : START[l] + kn * ksci].rearrange(
                    "b (k o) -> b k o", o=ksci
                )
                y_col = y[:, START[l] + kn * ksci : START[l] + kn * ksci + kn]
                ts = TS[l]
                tl = min(TLOAD, TOKL[l] - t0)
                xt3 = xts[(l, t0)]
                for s0 in range(0, tl, ts):
                    sl = min(ts, tl - s0)          # tokens in subtile
                    tok = t0 + s0                  # layer-token index
                    b0 = tok // kn                 # first batch
                    nb = max(1, sl // kn)          # batches in subtile
                    k0 = tok - b0 * kn             # first k
                    ob = out_pool.tile([128, 4096], F16, tag="ob")
                    for oi, (o0, no) in enumerate(otiles):
                        ps = ps_pool.tile([128, OTILE], F32, tag="ps")
                        for dc in range(4):
                            nc.tensor.matmul(
                                out=ps[:sl, :no],
                                lhsT=xt3[:, dc, s0 : s0 + sl],
                                rhs=wt3[l][:, dc, o0 : o0 + no],
                                start=(dc == 0),
                                stop=(dc == 3),
                            )
                        nc.vector.tensor_add(
                            out=ob[:sl, o0 : o0 + no],
                            in0=ps[:sl, :no],
                            in1=bb[:sl, BBOFF[l] + o0 : BBOFF[l] + o0 + no],
                        )
                    if l != 3:
                        c = CBASE[l] + tok // ts
                        nc.scalar.copy(
                            out=col_acc[:sl, c : c + 1],
                            in_=ob[:sl, ksci : ksci + 1],
                        )
                    if (l, t0, s0) == (2, 0, 0):
                        # release the gated W3 load now that the warm-up
                        # ramp's read burst is past: the copy writes into
                        # the tile W3's DMA overwrites, so the DMA is
                        # ordered after it (real WAW dependency)
                        nc.gpsimd.tensor_copy(
                            out=wt3_tile[0:1, 0:1], in_=ob[0:1, 0:1]
                        )
                        nc.gpsimd.dma_start(out=wt3_tile[:], in_=WT[3][:, :])
                        load_bb(3)
                        nc.gpsimd.dma_start(out=idn[:], in_=IDN[:, :])
                    # store per batch: [nk, ksci] rows contiguous in y
                    nk = min(kn, sl)
                    for bi in range(nb):
                        nc.scalar.dma_start(
                            out=y_main[b0 + bi, k0 : k0 + nk, :],
                            in_=ob[bi * nk : bi * nk + nk, :ksci],
                        )
                if l == 3:
                    # transposed bias-column pass: [1, token] rows
                    for c0 in range(0, tl, OTILE):
                        cl = min(OTILE, tl - c0)
                        pc = ps_pool.tile([128, OTILE], F32, tag="ps")
                        for dc in range(4):
                            nc.tensor.matmul(
                                out=pc[:1, :cl],
                                lhsT=wt3[3][:, dc, ksci : ksci + 1],
                                rhs=xt3[:, dc, c0 : c0 + cl],
                                start=(dc == 0),
                                stop=(dc == 3),
                            )
                        oc = ocol_pool.tile([1, OTILE], F16, tag="oc")
                        nc.any.tensor_scalar_add(
                            out=oc[:1, :cl],
                            in0=pc[:1, :cl],
                            scalar1=bcol[0:1, 3:4],
                        )
                        cb0 = (t0 + c0) // kn
                        for bi in range(cl // kn):
                            nc.sync.dma_start(
                                out=y_col[cb0 + bi, :],
                                in_=oc[0:1, bi * kn : (bi + 1) * kn],
                            )

            # transpose col_acc -> acc_t and store folded bias columns
            pst = pst_pool.tile([64, 128], F16, tag="pst")
            nc.tensor.transpose(out=pst[:, :], in_=col_acc[:, 0:64], identity=idn[:])
            nc.any.tensor_copy(out=acc_t[:, :], in_=pst[:, :])

            def col_view(l):
                s = START[l] + KN[l] * KSCI[l]
                return y[:, s : s + KN[l]]

            # L1: row b <-> batch b             [16,128] <- [16,128]
            nc.sync.dma_start(out=col_view(1), in_=acc_t[CBASE[1] : CBASE[1] + 16, :])
            # L2: rows 2b,2b+1 <-> batch b      [16,256] <- [32,128]
            nc.sync.dma_start(out=col_view(2), in_=acc_t[CBASE[2] : CBASE[2] + 32, :])
            # L0: row halves <-> batches        [16,64] <- [8,128]
            nc.sync.dma_start(out=col_view(0), in_=acc_t[CBASE[0] : CBASE[0] + 8, :])
            # L4: row 0 tokens 0-119 (b 0-11), row 1 tokens 0-39 (b 12-15)
            c4 = col_view(4)
            nc.sync.dma_start(out=c4[0:12, :], in_=acc_t[CBASE[4] : CBASE[4] + 1, 0:120])
            nc.sync.dma_start(out=c4[12:16, :], in_=acc_t[CBASE[4] + 1 : CBASE[4] + 2, 0:40])
    nc.compile()
    return nc


def _prep_inputs(inputs):
    x = np.asarray(inputs["x"], dtype=np.float32)
    xb = x.astype(ml_dtypes.bfloat16)
    in_maps = []
    # shared across cores
    shared = {}
    for l in range(5):
        W = np.asarray(inputs[f"W{l}"], dtype=np.float32)  # [IDIM, D]
        # d-interleaved: packed[p, c*IDIM+o] = W[o, c*128+p]
        wp = W.T.astype(ml_dtypes.bfloat16)                # [D, IDIM]
        wp = wp.reshape(4, 128, IDIM[l]).transpose(1, 0, 2).reshape(128, -1)
        shared[f"WT{l}"] = np.ascontiguousarray(wp)
    bbvec = np.concatenate(
        [np.asarray(inputs[f"b{l}"], dtype=np.float32) for l in range(5)]
    )
    shared["BB"] = np.ascontiguousarray(
        np.broadcast_to(bbvec.astype(ml_dtypes.bfloat16), (128, BBTOT))
    )
    bcol = np.zeros((1, 8), np.float32)
    for l in range(5):
        bcol[0, l] = np.asarray(inputs[f"b{l}"], dtype=np.float32)[KSCI[l]]
    shared["BCOL"] = bcol
    shared["IDN"] = np.eye(128, dtype=np.float16)
    off = np.cumsum([0] + KN).tolist()
    for c in range(N_CORES):
        xc = xb[c * BPC : (c + 1) * BPC]  # [16, 714, 512] bf16
        parts = [
            np.transpose(xc[:, off[l] : off[l] + KN[l]], (2, 0, 1)).reshape(D, -1)
            for l in range(5)
        ]
        xT = np.concatenate(parts, axis=1)  # [512, 11424] d-major
        # per-chunk contiguous packing: [128, 4*tl] blocks, d-chunk-major
        blocks = []
        for l in range(5):
            for t0 in range(0, TOKL[l], TLOAD):
                tl = min(TLOAD, TOKL[l] - t0)
                blk = xT[:, XOFF[l] + t0 : XOFF[l] + t0 + tl]
                blocks.append(
                    blk.reshape(4, 128, tl).transpose(1, 0, 2).reshape(128, -1)
                )
        in_maps.append({"xP": np.ascontiguousarray(np.concatenate(blocks, axis=1)),
                        **shared})
    return in_maps


def kernel(**inputs):
    global last_results
    if "nc" not in _cache:
        _cache["nc"] = _build_bass()
    nc = _cache["nc"]
    in_maps = _prep_inputs(inputs)
    res = run_bass_kernel_spmd(nc, in_maps, list(range(N_CORES)))
    last_results = res
    y = np.concatenate(
        [res.results[c]["y"].astype(np.float32) for c in range(N_CORES)], axis=0
    )
    return y


# revision 36
# speedup vs baseline: 1.0733x; 1.0102x over previous
"""Trainium2 Bass kernel for nn_DebedderNeuronGroup_index.

Math (per layer l, with kn=KN[l], ksci=KS[l]*CI[l], i_dim=ksci+1):
    out[b, k, o] = sum_d x[b, off_l + k, d] * W_l[o, d] + b_l[o]
    y[b, S_l + k*ksci + o] = out[b, k, o]          for o <  ksci
    y[b, S_l + kn*ksci + k] = out[b, k, ksci]      (bias column tail block)
The five layers' outputs exactly tile y's 1,422,218 columns, so every
element of y is written exactly once (pure permutation, no accumulation).

Strategy: pure data parallelism over batch (16 per core, 8 cores).
Host pre-packs x per 1024-token chunk as [128, 4*tl] (d-chunk-major,
contiguous 8KB per partition => ~250GB/s chunk loads vs ~100GB/s for
the strided layout) and W as d-interleaved [128, 4*i_dim], all bf16.
Per 128-token subtile: tokens on PSUM partitions, o on the free dim, so
every HBM store is a [tokens, o] tile whose rows are contiguous in y.

Work order: both L1 chunks first (dense 512-wide matmuls warm the PE
HAM clock gate and cover the early HBM read burst), then L2 chunks 0-2,
L0 and L4 mid-stream (their many tiny stores overlap compute), then L3,
with L2's last chunk at the end (so the kernel tail is one small 0.3MB
store instead of L3's 1MB or L0/L4's 32 tiny DMAs).
All x chunks load up front on the SP ring and stay resident; W1/W2 on
the ACT ring ahead of the y stores; W3's 4MB load rides the gpsimd ring
but is gated behind L2-chunk0's first output tile so it can't steal HBM
read bandwidth from the x stream during the warm-up ramp.

The bias column (o == ksci) is folded into the last o-tile for layers
0/1/2/4 (free: their i_dim % 512 != 0); only layer 3 (4096 = 8*512)
keeps the separate M=1 transposed column pass. Folded bias columns are
extracted into col_acc (ACT engine), PE-transposed once at the end,
and stored with 6 batched DMAs.
"""

import numpy as np
import ml_dtypes

import concourse.bass as bass
import concourse.mybir as mybir
from concourse import bacc
from concourse.tile import TileContext
from concourse.bass_utils import run_bass_kernel_spmd

# ---------------------------------------------------------------- constants
N_CORES = 8
B = 128
BPC = B // N_CORES            # batches per core = 16
D = 512
KN = [64, 128, 256, 256, 10]
KSCI = [27, 576, 1152, 4096, 256]
IDIM = [k + 1 for k in KSCI]
START = [0, 1792, 75648, 370816, 1419648]
I_TOTAL = 1422218
TOK = sum(KN)                 # 714 tokens per batch
TOKL = [BPC * k for k in KN]  # tokens per core per layer
XOFF = np.cumsum([0] + TOKL).tolist()   # token offset per layer in xT
NTOK = XOFF[-1]               # 11424
BBOFF = np.cumsum([0] + IDIM).tolist()  # bias-broadcast offset per layer
BBTOT = BBOFF[-1]             # 6112
TLOAD = 1024                  # tokens per x DMA chunk
OTILE = 512                   # matmul moving free dim / PSUM bank
BF16 = mybir.dt.bfloat16
F16 = mybir.dt.float16
F32 = mybir.dt.float32

TS = {l: (128 if KN[l] >= 128 else (128 // KN[l]) * KN[l]) for l in range(5)}
# work items: (layer, chunk t0). Both L1 chunks first: dense 512-wide
# matmuls warm the PE HAM clock gate and give ~16us of covering work
# while the early HBM read burst (x + W2/W3 tables) streams in. L0/L4
# sit mid-stream where their many tiny stores overlap compute; L2's
# last chunk runs last so the kernel tail is one small 0.3MB store.
WORK = ([(1, 0), (1, TLOAD), (2, 0), (2, TLOAD), (2, 2 * TLOAD), (0, 0), (4, 0)]
        + [(3, t) for t in range(0, TOKL[3], TLOAD)]
        + [(2, 3 * TLOAD)])
# col_acc column base per layer (layer 3 uses the transposed pass)
NSUB = {l: (TOKL[l] + TS[l] - 1) // TS[l] for l in (0, 1, 2, 4)}
CBASE = {1: 0, 2: NSUB[1], 0: NSUB[1] + NSUB[2], 4: NSUB[1] + NSUB[2] + NSUB[0]}

_cache = {}
last_results = None


def _build_bass():
    nc = bacc.Bacc(
        "TRN2", target_bir_lowering=False, debug=False, num_devices=N_CORES
    )
    xP = nc.declare_dram_parameter("xP", [128, 4 * NTOK], BF16, isOutput=False)
    WT = [
        nc.declare_dram_parameter(f"WT{l}", [128, 4 * IDIM[l]], BF16, isOutput=False)
        for l in range(5)
    ]
    BB = nc.declare_dram_parameter("BB", [128, BBTOT], BF16, isOutput=False)
    BCOL = nc.declare_dram_parameter("BCOL", [1, 8], F32, isOutput=False)
    IDN = nc.declare_dram_parameter("IDN", [128, 128], F16, isOutput=False)
    y = nc.declare_dram_parameter("y", [BPC, I_TOTAL], F16, isOutput=True)

    with TileContext(nc) as tc:
        with (
            tc.tile_pool(name="wt", bufs=1) as wt_pool,
            tc.tile_pool(name="bias", bufs=1) as bias_pool,
            tc.tile_pool(name="x", bufs=12) as x_pool,
            tc.tile_pool(name="out", bufs=4) as out_pool,
            tc.tile_pool(name="ocol", bufs=4) as ocol_pool,
            tc.tile_pool(name="ps", bufs=7, space="PSUM") as ps_pool,
            tc.tile_pool(name="pst", bufs=1, space="PSUM") as pst_pool,
        ):
            bb = bias_pool.tile([128, BBTOT], BF16, tag="bb")
            idn = bias_pool.tile([128, 128], F16, tag="idn")
            col_acc = bias_pool.tile([128, 64], F16, tag="cacc")
            acc_t = bias_pool.tile([64, 128], F16, tag="accT")
            bcol = bias_pool.tile([1, 8], F32, tag="bcol")
            gate = bias_pool.tile([1, 8], F16, tag="gate")
            nc.gpsimd.memset(col_acc[:, :], 0.0)

            wt3 = {}

            def load_w(l, eng):
                t = wt_pool.tile([128, 4 * IDIM[l]], BF16, tag=f"wt{l}")
                eng.dma_start(out=t[:], in_=WT[l][:, :])
                wt3[l] = t[:].rearrange("p (c o) -> p c o", c=4)

            def load_bb(l):
                nc.gpsimd.dma_start(
                    out=bb[:, BBOFF[l] : BBOFF[l] + IDIM[l]],
                    in_=BB[:, BBOFF[l] : BBOFF[l] + IDIM[l]],
                )

            # W1 gates the first matmul; W1+W2 finish on the ACT HW ring
            # (~12us) before the y stores start queueing behind them.
            load_w(1, nc.scalar)
            load_w(2, nc.scalar)
            # gpsimd SW ring: bias slices + small tables in consumption
            # order; W3's 4MB lands mid-kernel, well before layer 3.
            nc.gpsimd.dma_start(out=bcol[:], in_=BCOL[:, :])
            load_bb(1)
            load_w(0, nc.gpsimd)
            load_bb(0)
            load_w(4, nc.gpsimd)
            load_bb(4)
            load_bb(2)
            # W3's tile exists up front; its 4MB DMA is released below via
            # a write into the tile (WAW ordering — the Tile scheduler
            # ignores emission order, so this is the only way to defer it).
            wt3_tile = wt_pool.tile([128, 4 * IDIM[3]], BF16, tag="wt3")
            wt3[3] = wt3_tile[:].rearrange("p (c o) -> p c o", c=4)

            # All x chunks up front, in work order. Each chunk is
            # host-packed contiguous ([128, 4*tl], d-chunk-major with
            # stride tl). The first chunk is split so the first matmuls
            # only wait on 256 tokens; two L2 chunks go via the ACT ring
            # to spread early HBM read demand.
            xts = {}
            for n, (l, t0) in enumerate(WORK):
                tl = min(TLOAD, TOKL[l] - t0)
                xt = x_pool.tile([128, 4 * TLOAD], BF16, tag="xt")
                fo = 4 * (XOFF[l] + t0)
                xt3 = xt[:, : 4 * tl].rearrange("p (c t) -> p c t", c=4)
                src3 = xP[:, fo : fo + 4 * tl].rearrange("p (c t) -> p c t", c=4)
                nc.sync.dma_start(out=xt3[:, :, :], in_=src3[:, :, :])
                xts[(l, t0)] = xt3

            for l, t0 in WORK:
                kn, ksci, idim = KN[l], KSCI[l], IDIM[l]
                ocols = idim if l != 3 else ksci
                otiles = [
                    (o0, min(OTILE, ocols - o0)) for o0 in range(0, ocols, OTILE)
                ]
                y_main = y[:, START[l] : START[l] + kn * ksci].rearrange(
                    "b (k o) -> b k o", o=ksci
                )
                y_col = y[:, START[l] + kn * ksci : START[l] + kn * ksci + kn]
                ts = TS[l]
                tl = min(TLOAD, TOKL[l] - t0)
                xt3 = xts[(l, t0)]
                for s0 in range(0, tl, ts):
                    sl = min(ts, tl - s0)          # tokens in subtile
                    tok = t0 + s0                  # layer-token index
                    b0 = tok // kn                 # first batch
                    nb = max(1, sl // kn)          # batches in subtile
                    k0 = tok - b0 * kn             # first k
                    ob = out_pool.tile([128, 4096], F16, tag="ob")
                    for oi, (o0, no) in enumerate(otiles):
                        ps = ps_pool.tile([128, OTILE], F32, tag="ps")
                        for dc in range(4):
                            nc.tensor.matmul(
                                out=ps[:sl, :no],
                                lhsT=xt3[:, dc, s0 : s0 + sl],
                                rhs=wt3[l][:, dc, o0 : o0 + no],
                                start=(dc == 0),
                                stop=(dc == 3),
                            )
                        nc.vector.tensor_add(
                            out=ob[:sl, o0 : o0 + no],
                            in0=ps[:sl, :no],
                            in1=bb[:sl, BBOFF[l] + o0 : BBOFF[l] + o0 + no],
                        )
                    if l != 3:
                        c = CBASE[l] + tok // ts
                        nc.scalar.copy(
                            out=col_acc[:sl, c : c + 1],
                            in_=ob[:sl, ksci : ksci + 1],
                        )
                    if (l, t0, s0) == (2, 2 * TLOAD, 0):
                        # release the gated W3 load now that the warm-up
                        # ramp's read burst is past: the copy writes into
                        # the tile W3's DMA overwrites, so the DMA is
                        # ordered after it (real WAW dependency)
                        nc.gpsimd.tensor_copy(
                            out=wt3_tile[0:1, 0:1], in_=ob[0:1, 0:1]
                        )
                        nc.gpsimd.dma_start(out=wt3_tile[:], in_=WT[3][:, :])
                        load_bb(3)
                        nc.gpsimd.dma_start(out=idn[:], in_=IDN[:, :])
                    # store per batch: [nk, ksci] rows contiguous in y
                    nk = min(kn, sl)
                    for bi in range(nb):
                        nc.scalar.dma_start(
                            out=y_main[b0 + bi, k0 : k0 + nk, :],
                            in_=ob[bi * nk : bi * nk + nk, :ksci],
                        )
                if l == 3:
                    # transposed bias-column pass: [1, token] rows
                    for c0 in range(0, tl, OTILE):
                        cl = min(OTILE, tl - c0)
                        pc = ps_pool.tile([128, OTILE], F32, tag="ps")
                        for dc in range(4):
                            nc.tensor.matmul(
                                out=pc[:1, :cl],
                                lhsT=wt3[3][:, dc, ksci : ksci + 1],
                                rhs=xt3[:, dc, c0 : c0 + cl],
                                start=(dc == 0),
                                stop=(dc == 3),
                            )
                        oc = ocol_pool.tile([1, OTILE], F16, tag="oc")
                        nc.any.tensor_scalar_add(
                            out=oc[:1, :cl],
                            in0=pc[:1, :cl],
                            scalar1=bcol[0:1, 3:4],
                        )
                        cb0 = (t0 + c0) // kn
                        for bi in range(cl // kn):
                            nc.sync.dma_start(
                                out=y_col[cb0 + bi, :],
                                in_=oc[0:1, bi * kn : (bi + 1) * kn],
                            )

            # transpose col_acc -> acc_t and store folded bias columns
            pst = pst_pool.tile([64, 128], F16, tag="pst")
            nc.tensor.transpose(out=pst[:, :], in_=col_acc[:, 0:64], identity=idn[:])
            nc.any.tensor_copy(out=acc_t[:, :], in_=pst[:, :])

            def col_view(l):
                s = START[l] + KN[l] * KSCI[l]
                return y[:, s : s + KN[l]]

            # L1: row b <-> batch b             [16,128] <- [16,128]
            nc.sync.dma_start(out=col_view(1), in_=acc_t[CBASE[1] : CBASE[1] + 16, :])
            # L2: rows 2b,2b+1 <-> batch b      [16,256] <- [32,128]
            nc.sync.dma_start(out=col_view(2), in_=acc_t[CBASE[2] : CBASE[2] + 32, :])
            # L0: row halves <-> batches        [16,64] <- [8,128]
            nc.sync.dma_start(out=col_view(0), in_=acc_t[CBASE[0] : CBASE[0] + 8, :])
            # L4: row 0 tokens 0-119 (b 0-11), row 1 tokens 0-39 (b 12-15)
            c4 = col_view(4)
            nc.sync.dma_start(out=c4[0:12, :], in_=acc_t[CBASE[4] : CBASE[4] + 1, 0:120])
            nc.sync.dma_start(out=c4[12:16, :], in_=acc_t[CBASE[4] + 1 : CBASE[4] + 2, 0:40])
    nc.compile()
    return nc


def _prep_inputs(inputs):
    x = np.asarray(inputs["x"], dtype=np.float32)
    xb = x.astype(ml_dtypes.bfloat16)
    in_maps = []
    # shared across cores
    shared = {}
    for l in range(5):
        W = np.asarray(inputs[f"W{l}"], dtype=np.float32)  # [IDIM, D]
        # d-interleaved: packed[p, c*IDIM+o] = W[o, c*128+p]
        wp = W.T.astype(ml_dtypes.bfloat16)                # [D, IDIM]
        wp = wp.reshape(4, 128, IDIM[l]).transpose(1, 0, 2).reshape(128, -1)
        shared[f"WT{l}"] = np.ascontiguousarray(wp)
    bbvec = np.concatenate(
        [np.asarray(inputs[f"b{l}"], dtype=np.float32) for l in range(5)]
    )
    shared["BB"] = np.ascontiguousarray(
        np.broadcast_to(bbvec.astype(ml_dtypes.bfloat16), (128, BBTOT))
    )
    bcol = np.zeros((1, 8), np.float32)
    for l in range(5):
        bcol[0, l] = np.asarray(inputs[f"b{l}"], dtype=np.float32)[KSCI[l]]
    shared["BCOL"] = bcol
    shared["IDN"] = np.eye(128, dtype=np.float16)
    off = np.cumsum([0] + KN).tolist()
    for c in range(N_CORES):
        xc = xb[c * BPC : (c + 1) * BPC]  # [16, 714, 512] bf16
        parts = [
            np.transpose(xc[:, off[l] : off[l] + KN[l]], (2, 0, 1)).reshape(D, -1)
            for l in range(5)
        ]
        xT = np.concatenate(parts, axis=1)  # [512, 11424] d-major
        # per-chunk contiguous packing: [128, 4*tl] blocks, d-chunk-major
        blocks = []
        for l in range(5):
            for t0 in range(0, TOKL[l], TLOAD):
                tl = min(TLOAD, TOKL[l] - t0)
                blk = xT[:, XOFF[l] + t0 : XOFF[l] + t0 + tl]
                blocks.append(
                    blk.reshape(4, 128, tl).transpose(1, 0, 2).reshape(128, -1)
                )
        in_maps.append({"xP": np.ascontiguousarray(np.concatenate(blocks, axis=1)),
                        **shared})
    return in_maps


def kernel(**inputs):
    global last_results
    if "nc" not in _cache:
        _cache["nc"] = _build_bass()
    nc = _cache["nc"]
    in_maps = _prep_inputs(inputs)
    res = run_bass_kernel_spmd(nc, in_maps, list(range(N_CORES)))
    last_results = res
    y = np.concatenate(
        [res.results[c]["y"].astype(np.float32) for c in range(N_CORES)], axis=0
    )
    return y
